# revision 1
# baseline (speedup 1.0000x reference)
"""AddLinearAttention TRN2 kernel — 8-core data-parallel over batch.

B=16, C=128, H=W=96. Per core: 2 batches. Depthwise convs:
- q/k/v 3x3: PE fp8 (3 DoubleRow pairs + 3 singles) + bf16 residual matmul
  (re-running the qkvo 1x1 slice) accumulating into one PSUM group.
- lepe 5x5: cols +-1 on PE fp8 (2 DR pairs + 1 single per col); cols {0,+-2}
  as full-image DVE STT chains on a bf16 pad.
DR moving operands are contiguous full-padded-row runs (dx baked into the
start offset; pad columns accumulate garbage that is never read), split into
<=1024-element halves. Biases folded analytically into post-PSUM bias terms.
kv via tiled dma_start_transpose (token-major) + 72 accumulating matmuls.
"""

import os
from contextlib import ExitStack

import numpy as np
import ml_dtypes

import concourse.tile as tile
from concourse import bacc, mybir
from concourse._compat import with_exitstack
from concourse.bass_utils import run_bass_kernel_spmd

B, C, H, W = 16, 128, 96, 96
HW = H * W
NCORES = 8
BPC = B // NCORES
RPC = 4                     # rows per chunk (normal mm moving <= 512 elems)
NCH = H // RPC              # 24 chunks
PADX = 2
WP = 112                    # padded row stride (16B-aligned for fp8 DR)
RS = 3                      # leading row slack (>= maxdy+1 for offset>=0)
DRL = (RPC - 1) * 112 + 2 + 2 + 96   # DR run length (436): last row needs cols<=435
HA = H + RS + 5             # allocated pad rows
SCALE = C ** (-0.5)
S2 = SCALE / HW
ZS = SCALE / HW

F32 = mybir.dt.float32
BF16 = mybir.dt.bfloat16
FP8 = mybir.dt.float8e4
AF = mybir.ActivationFunctionType
ALU = mybir.AluOpType
DR = mybir.MatmulPerfMode.DoubleRow
E4 = ml_dtypes.float8_e4m3fn

# ---- static tap tables -------------------------------------------------
QKV_PAIRS = [(-1, 0, dx) for dx in (-1, 0, 1)]       # (dy1, dy2, dx)
QKV_SINGLES = [(1, dx) for dx in (-1, 0, 1)]
LEP_PE_PAIRS = ([(-2, -1, dx) for dx in (-2, -1, 0, 1, 2)]
                + [(0, 1, dx) for dx in (-2, -1, 0, 1, 2)])
LEP_PE_SINGLES = [(2, dx) for dx in (-2, -1, 0, 1, 2)]

NPAIR = 19
NSING = 14


def _dr_ap(pad, y0, dy1, dy2, dx):
    """[C, 2, RPC*WP] contiguous full-row run; dx baked into start offset.
    2*4*112 = 896 <= 1024 (DR moving element limit)."""
    base = pad[:].rearrange("p a b -> p (a b)")
    st = (RS + y0 + dy1) * WP + dx
    w = base[:, st : st + DRL]
    u = w.unsqueeze(1).copy()
    lst = u.ap
    lst[1] = [(dy2 - dy1) * WP, 2]
    u.ap = lst
    return u


def _win(pad, y0, dy, dx, rpc=RPC):
    return pad[:, RS + y0 + dy : RS + y0 + dy + rpc, PADX + dx : PADX + dx + W]


@with_exitstack
def _build(ctx: ExitStack, tc: tile.TileContext):
    nc = tc.nc
    x_d = nc.dram_tensor("x", [BPC, C, H, W], F32, kind="ExternalInput").ap()
    wqkvo_d = nc.dram_tensor("wqkvo_t", [C, 4 * C], F32, kind="ExternalInput").ap()
    wproj_d = nc.dram_tensor("wproj_t", [C, C], F32, kind="ExternalInput").ap()
    prs_d = nc.dram_tensor("drpairs", [C, NPAIR, 2, C], FP8, kind="ExternalInput").ap()
    sng_d = nc.dram_tensor("drsing", [C, NSING, C], FP8, kind="ExternalInput").ap()
    bias_d = nc.dram_tensor("biases", [C, 8], F32, kind="ExternalInput").ap()
    out_d = nc.dram_tensor("out", [BPC, C, H, W], F32, kind="ExternalOutput").ap()

    const = ctx.enter_context(tc.tile_pool(name="const", bufs=1))
    wq_sb = const.tile([C, 4 * C], BF16, tag="wq")
    wp_sb = const.tile([C, C], BF16, tag="wp")
    prs = const.tile([C, NPAIR, 2, C], FP8, tag="prs")
    sng = const.tile([C, NSING, C], FP8, tag="sng")
    bi = const.tile([C, 8], F32, tag="bi")
    ones_sb = const.tile([C, C], BF16, tag="ones")
    inv_sb = const.tile([C, C], BF16, tag="inv")
    kmr_sb = const.tile([C, C], BF16, tag="kmr")
    ks_parts = const.tile([C, NCH], F32, tag="ksp")
    ks_sb = const.tile([C, 1], F32, tag="ks")
    kv_sb = const.tile([C, C], BF16, tag="kv")

    nc.gpsimd.dma_start(out=wq_sb[:], in_=wqkvo_d[:])   # f32->bf16 via SWDGE
    nc.gpsimd.dma_start(out=wp_sb[:], in_=wproj_d[:])
    nc.sync.dma_start(out=prs[:], in_=prs_d[:])
    nc.sync.dma_start(out=sng[:], in_=sng_d[:])
    nc.sync.dma_start(out=bi[:], in_=bias_d[:])
    nc.vector.memset(ones_sb[:], 1.0)
    nc.vector.memset(inv_sb[:], 1.0 / C)

    bq, bk, bv, bo, bl, bp = (bi[:, i : i + 1] for i in range(6))

    pads = ctx.enter_context(tc.tile_pool(name="pads", bufs=1))
    qpad = pads.tile([C, HA, WP], FP8, tag="qpad")
    kpad = pads.tile([C, HA, WP], FP8, tag="kpad")
    vpad = pads.tile([C, HA, WP], FP8, tag="vpad")
    for t in (qpad, kpad, vpad):
        nc.vector.memset(t[:, 0:RS, :], 0.0)                  # top slack rows
        nc.vector.memset(t[:, RS + H :, :], 0.0)              # bottom slack rows
        nc.vector.memset(t[:, RS : RS + H, 0:PADX], 0.0)      # left cols
        nc.vector.memset(t[:, RS : RS + H, PADX + W :], 0.0)  # right cols

    flats = ctx.enter_context(tc.tile_pool(name="flats", bufs=1))
    xin = ctx.enter_context(tc.tile_pool(name="xin", bufs=3))

    for b in [bb for _ in range(int(os.environ.get("KERNEL_REPEAT", "1")))
              for bb in range(BPC)]:
        qh = flats.tile([C, H, W], BF16, tag="qh")
        kh = flats.tile([C, H, W], BF16, tag="kh")
        vh = flats.tile([C, H, W], BF16, tag="vh")
        of = flats.tile([C, H, W], BF16, tag="of")
        lep = flats.tile([C, H, W], BF16, tag="lep")

        # ---- phase 1 (qkvo + pad emissions) fused with phase 2 (taps), lead 1
        # kv transposes+matmuls interleave as token rows complete.
        xtiles = {}
        accs = {}
        kf = kh[:].rearrange("p a b -> p (a b)")
        vf = vh[:].rearrange("p a b -> p (a b)")
        NJG = 9
        kv_ready = {}
        for jg in range(NJG):
            r_ready = -(-1024 * (jg + 1) // (RPC * W)) - 1   # ceil div - 1
            kv_ready.setdefault(min(r_ready, NCH - 1), []).append(jg)
        ktf = flats.tile([C, 72, C], BF16, tag="ktf")
        vtf = flats.tile([C, 72, C], BF16, tag="vtf")
        with tc.tile_pool(name="ps1", bufs=1, space="PSUM") as ps1, \
             tc.tile_pool(name="accA", bufs=2, space="PSUM") as accA, \
             tc.tile_pool(name="accV", bufs=2, space="PSUM") as accV, \
             tc.tile_pool(name="accL", bufs=1, space="PSUM") as accL, \
             tc.tile_pool(name="etmp", bufs=2) as etmp:
            for rr in range(NCH + 1):
                if rr < NCH:
                    y0 = rr * RPC
                    if rr % 3 == 0:
                        xg = xin.tile([C, 3 * RPC, W], BF16, tag="x")
                        nc.gpsimd.dma_start(
                            out=xg[:], in_=x_d[b, :, y0 : y0 + 3 * RPC, :])
                        for i in range(3):
                            xtiles[rr + i] = xg[:, i * RPC : (i + 1) * RPC, :]
                    xc = xtiles[rr]
                    cacc = []
                    for gi, (pad, pool) in enumerate(
                        ((qpad, accA), (kpad, accA), (vpad, accV))
                    ):
                        pa = pool.tile([C, RPC, WP], F32, tag=f"acc{gi}")
                        cacc.append(pa)
                        nc.tensor.matmul(
                            pa[:, :, PADX : PADX + W],
                            wq_sb[:, gi * C : (gi + 1) * C], xc,
                            start=True, stop=False,
                        )
                        if gi == 2:
                            nc.vector.tensor_scalar(
                                _win(pad, y0, 0, 0), pa[:, :, PADX : PADX + W],
                                1.0, None, ALU.mult)
                        else:
                            nc.scalar.activation(
                                _win(pad, y0, 0, 0), pa[:, :, PADX : PADX + W],
                                AF.Copy)
                    accs[rr] = cacc
                    p = ps1.tile([C, RPC, W], F32, tag="p1")
                    nc.tensor.matmul(
                        p[:], wq_sb[:, 3 * C : 4 * C], xc, start=True, stop=True)
                    nc.vector.tensor_scalar_add(
                        of[:, y0 : y0 + RPC, :], p[:], bo)
                if rr < 1:
                    continue
                r = rr - 1
                y0 = r * RPC
                xtiles.pop(r, None)

                # --- q/k/v convs on PE: fp8 DR pairs + singles accumulate onto
                # the phase-1 1x1 result already sitting in the accumulator
                for gi, (pad, bb, dst) in enumerate(
                    ((qpad, bq, qh), (kpad, bk, kh), (vpad, bv, vh))
                ):
                    pa = accs[r][gi]
                    paf = pa[:].rearrange("p a b -> p (a b)")[:, 0:DRL]
                    for j, (dy1, dy2, dx) in enumerate(QKV_PAIRS):
                        nc.tensor.matmul(
                            paf, prs[:, gi * 3 + j],
                            _dr_ap(pad, y0, dy1, dy2, dx),
                            start=False, stop=False, perf_mode=DR)
                    for j, (dy, dx) in enumerate(QKV_SINGLES):
                        last = j == len(QKV_SINGLES) - 1
                        nc.tensor.matmul(
                            pa[:, :, PADX : PADX + W], sng[:, gi * 3 + j],
                            _win(pad, y0, dy, dx), start=False, stop=last)
                    pint = pa[:, :, PADX : PADX + W]
                    if gi < 2:
                        e = etmp.tile([C, RPC, W], BF16, tag=f"e{gi}")
                        nc.scalar.activation(e[:], pint, AF.Exp, bias=bb)
                        rl = etmp.tile([C, RPC, W], BF16, tag=f"rl{gi}")
                        nc.vector.tensor_scalar(rl[:], pint, bb, 0.0,
                                                ALU.add, ALU.max)
                        if gi == 1:
                            nc.vector.scalar_tensor_tensor(
                                dst[:, y0 : y0 + RPC, :], e[:], 1.0, rl[:],
                                ALU.min, ALU.add,
                                accum_out=ks_parts[:, r : r + 1])
                        else:
                            m1 = etmp.tile([C, RPC, W], BF16, tag="m1")
                            nc.gpsimd.tensor_scalar_min(m1[:], e[:], 1.0)
                            nc.gpsimd.tensor_tensor(
                                dst[:, y0 : y0 + RPC, :], m1[:], rl[:], ALU.add)
                    else:
                        nc.scalar.activation(
                            dst[:, y0 : y0 + RPC, :], pint, AF.Identity, bias=bb)

                # --- lepe PE part (cols +-1)
                pl = accL.tile([C, RPC, WP], F32, tag="accl")
                plf = pl[:].rearrange("p a b -> p (a b)")[:, 0:DRL]
                for j, (dy1, dy2, dx) in enumerate(LEP_PE_PAIRS):
                    nc.tensor.matmul(
                        plf, prs[:, 9 + j],
                        _dr_ap(vpad, y0, dy1, dy2, dx),
                        start=(j == 0), stop=False, perf_mode=DR)
                for j, (dy, dx) in enumerate(LEP_PE_SINGLES):
                    last = j == len(LEP_PE_SINGLES) - 1
                    nc.tensor.matmul(
                        pl[:, :, PADX : PADX + W], sng[:, 9 + j],
                        _win(vpad, y0, dy, dx), start=False, stop=last)
                nc.scalar.activation(lep[:, y0 : y0 + RPC, :],
                                     pl[:, :, PADX : PADX + W], AF.Identity, bias=bl)

                for jg in kv_ready.get(r, []):
                    nc.sync.dma_start_transpose(
                        out=ktf[:, jg * 8 : (jg + 1) * 8, :],
                        in_=kf[:, jg * 1024 : (jg + 1) * 1024])
                    nc.sync.dma_start_transpose(
                        out=vtf[:, jg * 8 : (jg + 1) * 8, :],
                        in_=vf[:, jg * 1024 : (jg + 1) * 1024])

        # ---- kv: 72 accumulating matmuls over buffered token-major tiles
        with tc.tile_pool(name="kvp", bufs=1, space="PSUM") as kvp:
            kvacc = kvp.tile([C, C], F32, tag="kvacc")
            for j in range(72):
                nc.tensor.matmul(kvacc[:], ktf[:, j], vtf[:, j],
                                 start=(j == 0), stop=(j == 71))
            nc.scalar.activation(kv_sb[:], kvacc[:], AF.Copy, scale=float(S2))

        # ---- k_mean -> replicated scaled lhsT
        nc.vector.tensor_reduce(ks_sb[:], ks_parts[:], axis=mybir.AxisListType.X,
                                op=ALU.add)
        nc.vector.tensor_scalar(kmr_sb[:], ones_sb[:], ks_sb[:], ZS,
                                ALU.mult, ALU.mult)

        # ---- phase 4: res per chunk
        with tc.tile_pool(name="ps4", bufs=2, space="PSUM") as ps4, \
             tc.tile_pool(name="psf", bufs=2, space="PSUM") as psf, \
             tc.tile_pool(name="etmp4", bufs=2) as etmp, \
             tc.tile_pool(name="ostg", bufs=2) as ostg:
            for r in range(NCH):
                y0 = r * RPC
                rq = qh[:, y0 : y0 + RPC, :]
                rv = vh[:, y0 : y0 + RPC, :]
                zp = ps4.tile([C, RPC, W], F32, tag="zp")
                nc.tensor.matmul(zp[:], kmr_sb[:], rq, start=True, stop=True)
                vb = ps4.tile([C, RPC, W], F32, tag="vb")
                nc.tensor.matmul(vb[:], inv_sb[:], rv, start=True, stop=True)
                rp = ps4.tile([C, RPC, W], F32, tag="rp")
                nc.tensor.matmul(rp[:], kv_sb[:], rq, start=True, stop=True)

                zs = etmp.tile([C, RPC, W], BF16, tag="zs")
                nc.scalar.activation(zs[:], zp[:], AF.Copy)
                rr2 = etmp.tile([C, RPC, W], BF16, tag="rr2")
                with nc.allow_low_precision(reason="1/z feeds bf16 elementwise"):
                    nc.vector.reciprocal(rr2[:], zs[:])
                vbs = etmp.tile([C, RPC, W], BF16, tag="vbs")
                nc.scalar.activation(vbs[:], vb[:], AF.Copy)
                t1 = etmp.tile([C, RPC, W], BF16, tag="t1")
                nc.vector.scalar_tensor_tensor(
                    t1[:], rr2[:], 1.0, rp[:], ALU.add, ALU.mult)
                t2 = etmp.tile([C, RPC, W], BF16, tag="t2")
                nc.vector.tensor_tensor(t2[:], zs[:], vbs[:], ALU.mult)
                r3 = etmp.tile([C, RPC, W], BF16, tag="r3")
                nc.vector.tensor_tensor(r3[:], t1[:], t2[:], ALU.subtract)
                r4 = etmp.tile([C, RPC, W], BF16, tag="r4")
                nc.vector.tensor_tensor(r4[:], r3[:], lep[:, y0 : y0 + RPC, :],
                                        ALU.add)
                r5 = etmp.tile([C, RPC, W], BF16, tag="r5")
                nc.gpsimd.tensor_tensor(r5[:], r4[:], of[:, y0 : y0 + RPC, :],
                                        ALU.mult)
                pp = psf.tile([C, RPC, W], F32, tag="pp")
                nc.tensor.matmul(pp[:], wp_sb[:], r5[:], start=True, stop=True)
                if r % 3 == 0:
                    og = ostg.tile([C, 3 * RPC, W], F32, tag="og")
                nc.scalar.activation(og[:, (r % 3) * RPC : (r % 3 + 1) * RPC, :],
                                     pp[:], AF.Identity, bias=bp)
                if r % 3 == 2:
                    nc.sync.dma_start(
                        out=out_d[b, :, y0 - 2 * RPC : y0 + RPC, :], in_=og[:])


_CACHE = {}


def _get_nc():
    if "nc" not in _CACHE:
        nc = bacc.Bacc("TRN2", target_bir_lowering=False, debug=False)
        with tile.TileContext(nc, pool_alloc_mode="queue") as tc:
            _build(tc)
        nc.compile()
        _CACHE["nc"] = nc
    return _CACHE["nc"]


def _diag(w):
    d = np.zeros((C, C), np.float32)
    np.fill_diagonal(d, w)
    return d


def kernel(**inputs) -> np.ndarray:
    x = np.asarray(inputs["x"], np.float32)
    w_qkvo = np.asarray(inputs["w_qkvo"], np.float32)[:, :, 0, 0]
    b_qkvo = np.asarray(inputs["b_qkvo"], np.float32)
    w_lepe = np.asarray(inputs["w_lepe"], np.float32)[:, 0]
    b_lepe = np.asarray(inputs["b_lepe"], np.float32)
    w_proj = np.asarray(inputs["w_proj"], np.float32)[:, :, 0, 0]
    b_proj = np.asarray(inputs["b_proj"], np.float32)
    w_q = np.asarray(inputs["w_q"], np.float32)[:, 0]
    b_q = np.asarray(inputs["b_q"], np.float32)
    w_k = np.asarray(inputs["w_k"], np.float32)[:, 0]
    b_k = np.asarray(inputs["b_k"], np.float32)
    w_v = np.asarray(inputs["w_v"], np.float32)[:, 0]
    b_v = np.asarray(inputs["b_v"], np.float32)

    pairs = np.zeros((NPAIR, C, 2, C), np.float32)
    sings = np.zeros((NSING, C, C), np.float32)
    for gi, wt in enumerate((w_q, w_k, w_v)):
        for j, (dy1, dy2, dx) in enumerate(QKV_PAIRS):
            pairs[gi * 3 + j, :, 0, :] = _diag(wt[:, dy1 + 1, dx + 1])
            pairs[gi * 3 + j, :, 1, :] = _diag(wt[:, dy2 + 1, dx + 1])
        for j, (dy, dx) in enumerate(QKV_SINGLES):
            sings[gi * 3 + j] = _diag(wt[:, dy + 1, dx + 1])
    for j, (dy1, dy2, dx) in enumerate(LEP_PE_PAIRS):
        pairs[9 + j, :, 0, :] = _diag(w_lepe[:, dy1 + 2, dx + 2])
        pairs[9 + j, :, 1, :] = _diag(w_lepe[:, dy2 + 2, dx + 2])
    for j, (dy, dx) in enumerate(LEP_PE_SINGLES):
        sings[9 + j] = _diag(w_lepe[:, dy + 2, dx + 2])

    bq0, bk0, bv0, bo0 = (b_qkvo[i * C : (i + 1) * C] for i in range(4))
    beff_q = bq0 * (1.0 + w_q.reshape(C, -1).sum(1)) + b_q
    beff_k = bk0 * (1.0 + w_k.reshape(C, -1).sum(1)) + b_k
    beff_v = bv0 * (1.0 + w_v.reshape(C, -1).sum(1)) + b_v
    beff_l = bv0 * w_lepe.reshape(C, -1).sum(1) + b_lepe
    biases = np.stack([beff_q, beff_k, beff_v, bo0, beff_l, b_proj,
                       np.zeros(C, np.float32), np.zeros(C, np.float32)], axis=1)

    shared = {
        "wqkvo_t": np.ascontiguousarray(w_qkvo.T),
        "wproj_t": np.ascontiguousarray(w_proj.T),
        "drpairs": np.ascontiguousarray(pairs.transpose(1, 0, 2, 3)).astype(E4),
        "drsing": np.ascontiguousarray(sings.transpose(1, 0, 2)).astype(E4),
        "biases": biases.astype(np.float32),
    }
    xb = x.reshape(NCORES, BPC, C, H, W)
    in_maps = [{"x": np.ascontiguousarray(xb[i]), **shared} for i in range(NCORES)]

    nc = _get_nc()
    _CACHE["last_in_maps"] = in_maps
    r = run_bass_kernel_spmd(
        nc, in_maps, core_ids=list(range(NCORES)),
        trace=bool(int(os.environ.get("KERNEL_TRACE", "0"))),
    )
    _CACHE["last_results"] = r
    out = np.stack([r.results[i]["out"] for i in range(NCORES)])
    return out.reshape(B, C, H, W)



# revision 2
# speedup vs baseline: 89.1698x; 89.1698x over previous
"""AddLinearAttention TRN2 kernel — 8-core data-parallel over batch.

B=16, C=128, H=W=96. Per core: 2 batches. Depthwise convs:
- q/k/v 3x3: center tap folded into the qkvo 1x1 stationary (per-channel
  column scale); remaining 8 taps as 3 fp8 DR pairs + 2 singles on PE,
  accumulating onto the 1x1 residual already in PSUM. Tap weights are
  divided by (1+w_center) since the pads hold the scaled 1x1 output.
- lepe 5x5: 10 DR pairs + 5 singles from vpad (weights divided by the
  v-center scale baked into vpad).
DR moving operands are contiguous full-padded-row runs (dx baked into the
start offset; pad columns accumulate garbage that is never read), split into
<=1024-element halves. Biases folded analytically into post-PSUM bias terms.
kv via tiled dma_start_transpose (token-major) + 72 accumulating matmuls;
the vtf tile carries a ones column so the same matmuls also produce the
k column-sums (k_mean) in fp32 for free.
"""

import os
from contextlib import ExitStack

import numpy as np
import ml_dtypes

import concourse.tile as tile
from concourse import bacc, mybir
from concourse._compat import with_exitstack
from concourse.bass_utils import run_bass_kernel_spmd

B, C, H, W = 16, 128, 96, 96
HW = H * W
NCORES = 8
BPC = B // NCORES
RPC = 4                     # rows per chunk (normal mm moving <= 512 elems)
NCH = H // RPC              # 24 chunks
PADX = 2
WP = 112                    # padded row stride (16B-aligned for fp8 DR)
RS = 3                      # leading row slack (>= maxdy+1 for offset>=0)
DRL = (RPC - 1) * 112 + 2 + 2 + 96   # DR run length (436): last row needs cols<=435
HA = H + RS + 5             # allocated pad rows
SCALE = C ** (-0.5)
S2 = SCALE / HW
ZS = SCALE / HW

F32 = mybir.dt.float32
BF16 = mybir.dt.bfloat16
FP8 = mybir.dt.float8e4
AF = mybir.ActivationFunctionType
ALU = mybir.AluOpType
DR = mybir.MatmulPerfMode.DoubleRow
E4 = ml_dtypes.float8_e4m3fn

# ---- static tap tables -------------------------------------------------
# center (0,0) folded into the 1x1 stationary; remaining 8 taps:
QKV_PAIRS = [(-1, 0, -1), (-1, 0, 1), (-1, 1, 0)]    # (dy1, dy2, dx)
QKV_SINGLES = [(1, -1), (1, 1)]                      # (dy, dx)
LEP_PE_PAIRS = ([(-2, -1, dx) for dx in (-2, -1, 0, 1, 2)]
                + [(0, 1, dx) for dx in (-2, -1, 0, 1, 2)])
LEP_PE_SINGLES = [(2, dx) for dx in (-2, -1, 0, 1, 2)]

NPAIR = 19
NSING = 11
KVN = 132                   # vtf free width: 128 data + 4 ones cols


def _dr_ap(pad, y0, dy1, dy2, dx):
    """[C, 2, RPC*WP] contiguous full-row run; dx baked into start offset.
    2*4*112 = 896 <= 1024 (DR moving element limit)."""
    base = pad[:].rearrange("p a b -> p (a b)")
    st = (RS + y0 + dy1) * WP + dx
    w = base[:, st : st + DRL]
    u = w.unsqueeze(1).copy()
    lst = u.ap
    lst[1] = [(dy2 - dy1) * WP, 2]
    u.ap = lst
    return u


def _win(pad, y0, dy, dx, rpc=RPC):
    return pad[:, RS + y0 + dy : RS + y0 + dy + rpc, PADX + dx : PADX + dx + W]


@with_exitstack
def _build(ctx: ExitStack, tc: tile.TileContext):
    nc = tc.nc
    x_d = nc.dram_tensor("x", [BPC, C, H, W], F32, kind="ExternalInput").ap()
    wqkvo_d = nc.dram_tensor("wqkvo_t", [C, 4 * C], F32, kind="ExternalInput").ap()
    wproj_d = nc.dram_tensor("wproj_t", [C, C], F32, kind="ExternalInput").ap()
    prs_d = nc.dram_tensor("drpairs", [C, NPAIR, 2, C], FP8, kind="ExternalInput").ap()
    sng_d = nc.dram_tensor("drsing", [C, NSING, C], FP8, kind="ExternalInput").ap()
    bias_d = nc.dram_tensor("biases", [C, 8], F32, kind="ExternalInput").ap()
    out_d = nc.dram_tensor("out", [BPC, C, H, W], F32, kind="ExternalOutput").ap()

    const = ctx.enter_context(tc.tile_pool(name="const", bufs=1))
    wq_sb = const.tile([C, 4 * C], BF16, tag="wq")
    wp_sb = const.tile([C, C], BF16, tag="wp")
    prs = const.tile([C, NPAIR, 2, C], FP8, tag="prs")
    sng = const.tile([C, NSING, C], FP8, tag="sng")
    bi = const.tile([C, 8], F32, tag="bi")
    ones_sb = const.tile([C, C], BF16, tag="ones")
    inv_sb = const.tile([C, C], BF16, tag="inv")
    kmr_sb = const.tile([C, C], BF16, tag="kmr")
    ks_sb = const.tile([C, 1], F32, tag="ks")
    kv_sb = const.tile([C, C], BF16, tag="kv")

    nc.gpsimd.dma_start(out=wq_sb[:], in_=wqkvo_d[:])   # f32->bf16 via SWDGE
    nc.gpsimd.dma_start(out=wp_sb[:], in_=wproj_d[:])
    nc.sync.dma_start(out=prs[:], in_=prs_d[:])
    nc.sync.dma_start(out=sng[:], in_=sng_d[:])
    nc.sync.dma_start(out=bi[:], in_=bias_d[:])
    nc.vector.memset(ones_sb[:], 1.0)
    nc.vector.memset(inv_sb[:], 1.0 / C)

    bq, bk, bv, bo, bl, bp = (bi[:, i : i + 1] for i in range(6))

    pads = ctx.enter_context(tc.tile_pool(name="pads", bufs=1))
    qpad = pads.tile([C, HA, WP], FP8, tag="qpad")
    kpad = pads.tile([C, HA, WP], FP8, tag="kpad")
    vpad = pads.tile([C, HA, WP], FP8, tag="vpad")
    for t in (qpad, kpad, vpad):
        nc.vector.memset(t[:, 0:RS, :], 0.0)                  # top slack rows
        nc.vector.memset(t[:, RS + H :, :], 0.0)              # bottom slack rows
        nc.vector.memset(t[:, RS : RS + H, 0:PADX], 0.0)      # left cols
        nc.vector.memset(t[:, RS : RS + H, PADX + W :], 0.0)  # right cols

    flats = ctx.enter_context(tc.tile_pool(name="flats", bufs=1))
    xin = ctx.enter_context(tc.tile_pool(name="xin", bufs=3))

    # token-major staging for kv; persistent so the ones cols are set once
    ktf = flats.tile([C, 72, C], BF16, tag="ktf")
    vtf = flats.tile([C, 72, KVN], BF16, tag="vtf")
    nc.vector.memset(vtf[:, :, C:KVN], 1.0)

    for b in [bb for _ in range(int(os.environ.get("KERNEL_REPEAT", "1")))
              for bb in range(BPC)]:
        qh = flats.tile([C, H, W], BF16, tag="qh")
        kh = flats.tile([C, H, W], BF16, tag="kh")
        vh = flats.tile([C, H, W], BF16, tag="vh")
        of = flats.tile([C, H, W], BF16, tag="of")
        lep = flats.tile([C, H, W], BF16, tag="lep")

        # ---- phase 1 (qkvo + pad emissions) fused with phase 2 (taps), lead 1
        # kv transposes+matmuls interleave as token rows complete.
        xtiles = {}
        accs = {}
        kf = kh[:].rearrange("p a b -> p (a b)")
        vf = vh[:].rearrange("p a b -> p (a b)")
        NJG = 9
        kv_ready = {}
        for jg in range(NJG):
            r_ready = -(-1024 * (jg + 1) // (RPC * W)) - 1   # ceil div - 1
            kv_ready.setdefault(min(r_ready, NCH - 1), []).append(jg)
        with tc.tile_pool(name="ps1", bufs=1, space="PSUM") as ps1, \
             tc.tile_pool(name="accA", bufs=2, space="PSUM") as accA, \
             tc.tile_pool(name="accV", bufs=2, space="PSUM") as accV, \
             tc.tile_pool(name="accL", bufs=1, space="PSUM") as accL, \
             tc.tile_pool(name="etmp", bufs=2) as etmp:
            for rr in range(NCH + 1):
                if rr < NCH:
                    y0 = rr * RPC
                    if rr % 3 == 0:
                        xg = xin.tile([C, 3 * RPC, W], BF16, tag="x")
                        nc.gpsimd.dma_start(
                            out=xg[:], in_=x_d[b, :, y0 : y0 + 3 * RPC, :])
                        for i in range(3):
                            xtiles[rr + i] = xg[:, i * RPC : (i + 1) * RPC, :]
                    xc = xtiles[rr]
                    cacc = []
                    for gi, (pad, pool) in enumerate(
                        ((qpad, accA), (kpad, accA), (vpad, accV))
                    ):
                        pa = pool.tile([C, RPC, WP], F32, tag=f"acc{gi}")
                        cacc.append(pa)
                        nc.tensor.matmul(
                            pa[:, :, PADX : PADX + W],
                            wq_sb[:, gi * C : (gi + 1) * C], xc,
                            start=True, stop=False,
                        )
                        if gi == 2:
                            nc.vector.tensor_scalar(
                                _win(pad, y0, 0, 0), pa[:, :, PADX : PADX + W],
                                1.0, None, ALU.mult)
                        else:
                            nc.scalar.activation(
                                _win(pad, y0, 0, 0), pa[:, :, PADX : PADX + W],
                                AF.Copy)
                    accs[rr] = cacc
                    p = ps1.tile([C, RPC, W], F32, tag="p1")
                    nc.tensor.matmul(
                        p[:], wq_sb[:, 3 * C : 4 * C], xc, start=True, stop=True)
                    nc.vector.tensor_scalar_add(
                        of[:, y0 : y0 + RPC, :], p[:], bo)
                if rr < 1:
                    continue
                r = rr - 1
                y0 = r * RPC
                xtiles.pop(r, None)

                # --- q/k/v convs on PE: fp8 DR pairs + singles accumulate onto
                # the phase-1 1x1 result already sitting in the accumulator
                for gi, (pad, bb, dst) in enumerate(
                    ((qpad, bq, qh), (kpad, bk, kh), (vpad, bv, vh))
                ):
                    pa = accs[r][gi]
                    paf = pa[:].rearrange("p a b -> p (a b)")[:, 0:DRL]
                    for j, (dy1, dy2, dx) in enumerate(QKV_PAIRS):
                        nc.tensor.matmul(
                            paf, prs[:, gi * 3 + j],
                            _dr_ap(pad, y0, dy1, dy2, dx),
                            start=False, stop=False, perf_mode=DR)
                    for j, (dy, dx) in enumerate(QKV_SINGLES):
                        last = j == len(QKV_SINGLES) - 1
                        nc.tensor.matmul(
                            pa[:, :, PADX : PADX + W], sng[:, gi * 2 + j],
                            _win(pad, y0, dy, dx), start=False, stop=last)
                    pint = pa[:, :, PADX : PADX + W]
                    if gi < 2:
                        e = etmp.tile([C, RPC, W], BF16, tag=f"e{gi}")
                        nc.scalar.activation(e[:], pint, AF.Exp, bias=bb)
                        rl = etmp.tile([C, RPC, W], BF16, tag=f"rl{gi}")
                        nc.vector.tensor_scalar(rl[:], pint, bb, 0.0,
                                                ALU.add, ALU.max)
                        m1 = etmp.tile([C, RPC, W], BF16, tag=f"m{gi}")
                        nc.gpsimd.tensor_scalar_min(m1[:], e[:], 1.0)
                        nc.gpsimd.tensor_tensor(
                            dst[:, y0 : y0 + RPC, :], m1[:], rl[:], ALU.add)
                    else:
                        nc.scalar.activation(
                            dst[:, y0 : y0 + RPC, :], pint, AF.Identity, bias=bb)

                # --- lepe PE part
                pl = accL.tile([C, RPC, WP], F32, tag="accl")
                plf = pl[:].rearrange("p a b -> p (a b)")[:, 0:DRL]
                for j, (dy1, dy2, dx) in enumerate(LEP_PE_PAIRS):
                    nc.tensor.matmul(
                        plf, prs[:, 9 + j],
                        _dr_ap(vpad, y0, dy1, dy2, dx),
                        start=(j == 0), stop=False, perf_mode=DR)
                for j, (dy, dx) in enumerate(LEP_PE_SINGLES):
                    last = j == len(LEP_PE_SINGLES) - 1
                    nc.tensor.matmul(
                        pl[:, :, PADX : PADX + W], sng[:, 6 + j],
                        _win(vpad, y0, dy, dx), start=False, stop=last)
                nc.scalar.activation(lep[:, y0 : y0 + RPC, :],
                                     pl[:, :, PADX : PADX + W], AF.Identity, bias=bl)

                for jg in kv_ready.get(r, []):
                    nc.sync.dma_start_transpose(
                        out=ktf[:, jg * 8 : (jg + 1) * 8, :],
                        in_=kf[:, jg * 1024 : (jg + 1) * 1024])
                    nc.sync.dma_start_transpose(
                        out=vtf[:, jg * 8 : (jg + 1) * 8, 0:C],
                        in_=vf[:, jg * 1024 : (jg + 1) * 1024])

        # ---- kv: 72 accumulating matmuls over buffered token-major tiles.
        # The ones column of vtf makes column C of the accumulator the
        # fp32 k column-sum (-> k_mean) at negligible extra cost.
        with tc.tile_pool(name="kvp", bufs=1, space="PSUM") as kvp:
            kvacc = kvp.tile([C, C + 1], F32, tag="kvacc")
            for j in range(72):
                nc.tensor.matmul(kvacc[:], ktf[:, j], vtf[:, j, 0 : C + 1],
                                 start=(j == 0), stop=(j == 71))
            nc.scalar.activation(kv_sb[:], kvacc[:, 0:C], AF.Copy, scale=float(S2))
            nc.scalar.activation(ks_sb[:], kvacc[:, C : C + 1], AF.Copy)

        # ---- k_mean -> replicated scaled lhsT
        nc.vector.tensor_scalar(kmr_sb[:], ones_sb[:], ks_sb[:], ZS,
                                ALU.mult, ALU.mult)

        # ---- phase 4: res per chunk
        with tc.tile_pool(name="ps4", bufs=2, space="PSUM") as ps4, \
             tc.tile_pool(name="psf", bufs=2, space="PSUM") as psf, \
             tc.tile_pool(name="etmp4", bufs=2) as etmp, \
             tc.tile_pool(name="ostg", bufs=2) as ostg:
            for r in range(NCH):
                y0 = r * RPC
                rq = qh[:, y0 : y0 + RPC, :]
                rv = vh[:, y0 : y0 + RPC, :]
                zp = ps4.tile([C, RPC, W], F32, tag="zp")
                nc.tensor.matmul(zp[:], kmr_sb[:], rq, start=True, stop=True)
                vb = ps4.tile([C, RPC, W], F32, tag="vb")
                nc.tensor.matmul(vb[:], inv_sb[:], rv, start=True, stop=True)
                rp = ps4.tile([C, RPC, W], F32, tag="rp")
                nc.tensor.matmul(rp[:], kv_sb[:], rq, start=True, stop=True)

                rr2 = etmp.tile([C, RPC, W], BF16, tag="rr2")
                with nc.allow_low_precision(reason="1/z feeds bf16 elementwise"):
                    nc.vector.reciprocal(rr2[:], zp[:])
                vbs = etmp.tile([C, RPC, W], BF16, tag="vbs")
                nc.scalar.activation(vbs[:], vb[:], AF.Copy)
                t1 = etmp.tile([C, RPC, W], BF16, tag="t1")
                nc.vector.scalar_tensor_tensor(
                    t1[:], rr2[:], 1.0, rp[:], ALU.add, ALU.mult)
                t2 = etmp.tile([C, RPC, W], BF16, tag="t2")
                nc.vector.tensor_tensor(t2[:], zp[:], vbs[:], ALU.mult)
                r3 = etmp.tile([C, RPC, W], BF16, tag="r3")
                nc.gpsimd.tensor_tensor(r3[:], t1[:], t2[:], ALU.subtract)
                r4 = etmp.tile([C, RPC, W], BF16, tag="r4")
                nc.gpsimd.tensor_tensor(r4[:], r3[:], lep[:, y0 : y0 + RPC, :],
                                        ALU.add)
                r5 = etmp.tile([C, RPC, W], BF16, tag="r5")
                nc.gpsimd.tensor_tensor(r5[:], r4[:], of[:, y0 : y0 + RPC, :],
                                        ALU.mult)
                pp = psf.tile([C, RPC, W], F32, tag="pp")
                nc.tensor.matmul(pp[:], wp_sb[:], r5[:], start=True, stop=True)
                if r % 3 == 0:
                    og = ostg.tile([C, 3 * RPC, W], F32, tag="og")
                nc.scalar.activation(og[:, (r % 3) * RPC : (r % 3 + 1) * RPC, :],
                                     pp[:], AF.Identity, bias=bp)
                if r % 3 == 2:
                    nc.sync.dma_start(
                        out=out_d[b, :, y0 - 2 * RPC : y0 + RPC, :], in_=og[:])


_CACHE = {}


def _get_nc():
    if "nc" not in _CACHE:
        nc = bacc.Bacc("TRN2", target_bir_lowering=False, debug=False)
        with tile.TileContext(nc, pool_alloc_mode="queue") as tc:
            _build(tc)
        nc.compile()
        _CACHE["nc"] = nc
    return _CACHE["nc"]


def _diag(w):
    d = np.zeros((C, C), np.float32)
    np.fill_diagonal(d, w)
    return d


def kernel(**inputs) -> np.ndarray:
    x = np.asarray(inputs["x"], np.float32)
    w_qkvo = np.asarray(inputs["w_qkvo"], np.float32)[:, :, 0, 0]
    b_qkvo = np.asarray(inputs["b_qkvo"], np.float32)
    w_lepe = np.asarray(inputs["w_lepe"], np.float32)[:, 0]
    b_lepe = np.asarray(inputs["b_lepe"], np.float32)
    w_proj = np.asarray(inputs["w_proj"], np.float32)[:, :, 0, 0]
    b_proj = np.asarray(inputs["b_proj"], np.float32)
    w_q = np.asarray(inputs["w_q"], np.float32)[:, 0]
    b_q = np.asarray(inputs["b_q"], np.float32)
    w_k = np.asarray(inputs["w_k"], np.float32)[:, 0]
    b_k = np.asarray(inputs["b_k"], np.float32)
    w_v = np.asarray(inputs["w_v"], np.float32)[:, 0]
    b_v = np.asarray(inputs["b_v"], np.float32)

    # center-tap folding: the 1x1 stationary columns for q/k/v are scaled by
    # (1 + w_center); the pads then hold the scaled 1x1 output, so every tap
    # weight that reads a pad is divided by that channel's center scale.
    cq = 1.0 + w_q[:, 1, 1]
    ck = 1.0 + w_k[:, 1, 1]
    cv = 1.0 + w_v[:, 1, 1]

    pairs = np.zeros((NPAIR, C, 2, C), np.float32)
    sings = np.zeros((NSING, C, C), np.float32)
    for gi, (wt, cs) in enumerate(((w_q, cq), (w_k, ck), (w_v, cv))):
        wt_s = wt / cs[:, None, None]
        for j, (dy1, dy2, dx) in enumerate(QKV_PAIRS):
            pairs[gi * 3 + j, :, 0, :] = _diag(wt_s[:, dy1 + 1, dx + 1])
            pairs[gi * 3 + j, :, 1, :] = _diag(wt_s[:, dy2 + 1, dx + 1])
        for j, (dy, dx) in enumerate(QKV_SINGLES):
            sings[gi * 2 + j] = _diag(wt_s[:, dy + 1, dx + 1])
    wl_s = w_lepe / cv[:, None, None]
    for j, (dy1, dy2, dx) in enumerate(LEP_PE_PAIRS):
        pairs[9 + j, :, 0, :] = _diag(wl_s[:, dy1 + 2, dx + 2])
        pairs[9 + j, :, 1, :] = _diag(wl_s[:, dy2 + 2, dx + 2])
    for j, (dy, dx) in enumerate(LEP_PE_SINGLES):
        sings[6 + j] = _diag(wl_s[:, dy + 2, dx + 2])

    wq_scaled = w_qkvo.copy()           # [4C, C_in]
    wq_scaled[0 * C : 1 * C] *= cq[:, None]
    wq_scaled[1 * C : 2 * C] *= ck[:, None]
    wq_scaled[2 * C : 3 * C] *= cv[:, None]

    bq0, bk0, bv0, bo0 = (b_qkvo[i * C : (i + 1) * C] for i in range(4))
    beff_q = bq0 * (1.0 + w_q.reshape(C, -1).sum(1)) + b_q
    beff_k = bk0 * (1.0 + w_k.reshape(C, -1).sum(1)) + b_k
    beff_v = bv0 * (1.0 + w_v.reshape(C, -1).sum(1)) + b_v
    beff_l = bv0 * w_lepe.reshape(C, -1).sum(1) + b_lepe
    biases = np.stack([beff_q, beff_k, beff_v, bo0, beff_l, b_proj,
                       np.zeros(C, np.float32), np.zeros(C, np.float32)], axis=1)

    shared = {
        "wqkvo_t": np.ascontiguousarray(wq_scaled.T),
        "wproj_t": np.ascontiguousarray(w_proj.T),
        "drpairs": np.ascontiguousarray(pairs.transpose(1, 0, 2, 3)).astype(E4),
        "drsing": np.ascontiguousarray(sings.transpose(1, 0, 2)).astype(E4),
        "biases": biases.astype(np.float32),
    }
    xb = x.reshape(NCORES, BPC, C, H, W)
    in_maps = [{"x": np.ascontiguousarray(xb[i]), **shared} for i in range(NCORES)]

    nc = _get_nc()
    _CACHE["last_in_maps"] = in_maps
    r = run_bass_kernel_spmd(
        nc, in_maps, core_ids=list(range(NCORES)),
        trace=bool(int(os.environ.get("KERNEL_TRACE", "0"))),
    )
    _CACHE["last_results"] = r
    out = np.stack([r.results[i]["out"] for i in range(NCORES)])
    return out.reshape(B, C, H, W)


# revision 17
# speedup vs baseline: 105.6960x; 1.1853x over previous
"""AddLinearAttention TRN2 kernel — 8-core data-parallel over batch.

B=16, C=128, H=W=96. Per core: 2 batches. Depthwise convs:
- q/k/v 3x3: center tap folded into the qkvo 1x1 stationary (per-channel
  column scale); remaining 8 taps as 3 fp8 DR pairs + 2 singles on PE,
  accumulating onto the 1x1 residual already in PSUM. Tap weights are
  divided by (1+w_center) since the pads hold the scaled 1x1 output.
- lepe 5x5: 10 DR pairs + 5 singles from vpad (weights divided by the
  v-center scale baked into vpad).
DR moving operands are contiguous full-padded-row runs (dx baked into the
start offset; pad columns accumulate garbage that is never read), split into
<=1024-element halves. Biases folded analytically into post-PSUM bias terms.
kv via tiled dma_start_transpose (token-major) + 72 accumulating matmuls;
the vtf tile carries a ones column so the same matmuls also produce the
k column-sums (k_mean) in fp32 for free.
"""

import os
from contextlib import ExitStack

import numpy as np
import ml_dtypes

import concourse.tile as tile
from concourse import bacc, mybir
from concourse._compat import with_exitstack
from concourse.bass_utils import run_bass_kernel_spmd

B, C, H, W = 16, 128, 96, 96
HW = H * W
NCORES = 8
BPC = B // NCORES
RPC = 4                     # rows per chunk (normal mm moving <= 512 elems)
NCH = H // RPC              # 24 chunks
PADX = 2
WP = 112                    # padded row stride (16B-aligned for fp8 DR)
RS = 3                      # leading row slack (>= maxdy+1 for offset>=0)
DRL = (RPC - 1) * 112 + 2 + 2 + 96   # DR run length (436): last row needs cols<=435
HA = H + RS + 5             # allocated pad rows
SCALE = C ** (-0.5)
S2 = SCALE / HW
ZS = SCALE / HW

F32 = mybir.dt.float32
BF16 = mybir.dt.bfloat16
FP8 = mybir.dt.float8e4
AF = mybir.ActivationFunctionType
ALU = mybir.AluOpType
DR = mybir.MatmulPerfMode.DoubleRow
E4 = ml_dtypes.float8_e4m3fn

# ---- static tap tables -------------------------------------------------
# center (0,0) folded into the 1x1 stationary; remaining 8 taps:
QKV_PAIRS = [(-1, 0, -1), (-1, 0, 1), (-1, 1, 0)]    # (dy1, dy2, dx)
QKV_SINGLES = [(1, -1), (1, 1)]                      # (dy, dx)
LEP_PE_PAIRS = ([(-2, -1, dx) for dx in (-2, -1, 0, 1, 2)]
                + [(0, 1, dx) for dx in (-2, -1, 0, 1, 2)])
LEP_PE_SINGLES = [(2, dx) for dx in (-2, -1, 0, 1, 2)]

NPAIR = 19
NSING = 11


def _dr_ap(pad, y0, dy1, dy2, dx):
    """[C, 2, RPC*WP] contiguous full-row run; dx baked into start offset.
    2*4*112 = 896 <= 1024 (DR moving element limit)."""
    base = pad[:].rearrange("p a b -> p (a b)")
    st = (RS + y0 + dy1) * WP + dx
    w = base[:, st : st + DRL]
    u = w.unsqueeze(1).copy()
    lst = u.ap
    lst[1] = [(dy2 - dy1) * WP, 2]
    u.ap = lst
    return u


def _win(pad, y0, dy, dx, rpc=RPC):
    return pad[:, RS + y0 + dy : RS + y0 + dy + rpc, PADX + dx : PADX + dx + W]


@with_exitstack
def _build(ctx: ExitStack, tc: tile.TileContext):
    nc = tc.nc
    drm = (mybir.MatmulPerfMode.DoubleRowSwInterleave
           if int(os.environ.get("KERNEL_DRSWI", "0")) else DR)
    x_d = nc.dram_tensor("x", [BPC, C, H, W], F32, kind="ExternalInput").ap()
    wqkvo_d = nc.dram_tensor("wqkvo_t", [C, 4 * C], F32, kind="ExternalInput").ap()
    wproj_d = nc.dram_tensor("wproj_t", [C, C], F32, kind="ExternalInput").ap()
    prs_d = nc.dram_tensor("drpairs", [C, NPAIR, 2, C], FP8, kind="ExternalInput").ap()
    sng_d = nc.dram_tensor("drsing", [C, NSING, C], FP8, kind="ExternalInput").ap()
    bias_d = nc.dram_tensor("biases", [C, 8], F32, kind="ExternalInput").ap()
    out_d = nc.dram_tensor("out", [BPC, C, H, W], F32, kind="ExternalOutput").ap()

    const = ctx.enter_context(tc.tile_pool(name="const", bufs=1))
    wq_sb = const.tile([C, 4 * C], BF16, tag="wq")
    wp_sb = const.tile([C, C], BF16, tag="wp")
    prs = const.tile([C, NPAIR, 2, C], FP8, tag="prs")
    sng = const.tile([C, NSING, C], FP8, tag="sng")
    bi = const.tile([C, 8], F32, tag="bi")
    ones_sb = const.tile([C, C], BF16, tag="ones")
    inv_sb = const.tile([C, C], BF16, tag="inv")
    kmr_sb = const.tile([C, C], BF16, tag="kmr")
    ks_parts = const.tile([C, NCH], F32, tag="ksp")
    ks_sb = const.tile([C, 1], F32, tag="ks")
    kv_sb = const.tile([C, C], BF16, tag="kv")

    nc.gpsimd.dma_start(out=wq_sb[:], in_=wqkvo_d[:])   # f32->bf16 via SWDGE
    nc.gpsimd.dma_start(out=wp_sb[:], in_=wproj_d[:])
    nc.sync.dma_start(out=prs[:], in_=prs_d[:])
    nc.sync.dma_start(out=sng[:], in_=sng_d[:])
    nc.sync.dma_start(out=bi[:], in_=bias_d[:])
    nc.vector.memset(ones_sb[:], 1.0)
    nc.vector.memset(inv_sb[:], 1.0 / C)

    bq, bk, bv, bo, bl, bp = (bi[:, i : i + 1] for i in range(6))

    pads = ctx.enter_context(tc.tile_pool(name="pads", bufs=1))
    qpad = pads.tile([C, HA, WP], FP8, tag="qpad")
    kpad = pads.tile([C, HA, WP], FP8, tag="kpad")
    vpad = pads.tile([C, HA, WP], FP8, tag="vpad")
    for t in (qpad, kpad, vpad):
        nc.vector.memset(t[:, 0:RS, :], 0.0)                  # top slack rows
        nc.vector.memset(t[:, RS + H :, :], 0.0)              # bottom slack rows
        nc.vector.memset(t[:, RS : RS + H, 0:PADX], 0.0)      # left cols
        nc.vector.memset(t[:, RS : RS + H, PADX + W :], 0.0)  # right cols

    flats = ctx.enter_context(tc.tile_pool(name="flats", bufs=1))
    xin = ctx.enter_context(tc.tile_pool(name="xin", bufs=3))

    # token-major staging for kv (persistent across batches)
    ktf = flats.tile([C, 72, C], BF16, tag="ktf")
    vtf = flats.tile([C, 72, C], BF16, tag="vtf")

    NQ = 4                      # quarter-image granularity for batch overlap
    CPQ = NCH // NQ             # chunks per quarter (6)
    QR = CPQ * RPC              # rows per quarter (24)

    for b in [bb for _ in range(int(os.environ.get("KERNEL_REPEAT", "1")))
              for bb in range(BPC)]:
        # quarter-split flats: WAR deps resolve per quarter, so the next
        # batch's phase 1 can overlap this batch's phase 4 tail.
        qh = [flats.tile([C, QR, W], BF16, tag=f"qh{i}", name=f"qh{i}") for i in range(NQ)]
        kh = [flats.tile([C, QR, W], BF16, tag=f"kh{i}", name=f"kh{i}") for i in range(NQ)]
        vh = [flats.tile([C, QR, W], BF16, tag=f"vh{i}", name=f"vh{i}") for i in range(NQ)]
        of = [flats.tile([C, QR, W], BF16, tag=f"of{i}", name=f"of{i}") for i in range(NQ)]
        lep = [flats.tile([C, QR, W], BF16, tag=f"lep{i}", name=f"lep{i}") for i in range(NQ)]

        def _rows(ts, r):
            ly = (r % CPQ) * RPC
            return ts[r // CPQ][:, ly : ly + RPC, :]

        # ---- phase 1 (qkvo + pad emissions) fused with phase 2 (taps), lead 1
        # kv transposes+matmuls interleave as quarters complete.
        xtiles = {}
        accs = {}
        with tc.tile_pool(name="ps1", bufs=1, space="PSUM") as ps1, \
             tc.tile_pool(name="accA", bufs=2, space="PSUM") as accA, \
             tc.tile_pool(name="accV", bufs=2, space="PSUM") as accV, \
             tc.tile_pool(name="accL", bufs=1, space="PSUM") as accL, \
             tc.tile_pool(name="etmp", bufs=2) as etmp:
            for rr in range(NCH + 1):
                if rr < NCH:
                    y0 = rr * RPC
                    if rr % 3 == 0:
                        xg = xin.tile([C, 3 * RPC, W], BF16, tag="x")
                        nc.gpsimd.dma_start(
                            out=xg[:], in_=x_d[b, :, y0 : y0 + 3 * RPC, :])
                        for i in range(3):
                            xtiles[rr + i] = xg[:, i * RPC : (i + 1) * RPC, :]
                    xc = xtiles[rr]
                    cacc = []
                    for gi, (pad, pool) in enumerate(
                        ((qpad, accA), (kpad, accA), (vpad, accV))
                    ):
                        pa = pool.tile([C, RPC, WP], F32, tag=f"acc{gi}")
                        cacc.append(pa)
                        nc.tensor.matmul(
                            pa[:, :, PADX : PADX + W],
                            wq_sb[:, gi * C : (gi + 1) * C], xc,
                            start=True, stop=False,
                        )
                        if gi == 2:
                            nc.vector.tensor_scalar(
                                _win(pad, y0, 0, 0), pa[:, :, PADX : PADX + W],
                                1.0, None, ALU.mult)
                        else:
                            nc.scalar.activation(
                                _win(pad, y0, 0, 0), pa[:, :, PADX : PADX + W],
                                AF.Copy)
                    accs[rr] = cacc
                    p = ps1.tile([C, RPC, W], F32, tag="p1")
                    nc.tensor.matmul(
                        p[:], wq_sb[:, 3 * C : 4 * C], xc, start=True, stop=True)
                    nc.vector.tensor_scalar_add(_rows(of, rr), p[:], bo)
                if rr < 1:
                    continue
                r = rr - 1
                y0 = r * RPC
                xtiles.pop(r, None)

                # --- q/k/v convs on PE: fp8 DR pairs + singles accumulate onto
                # the phase-1 1x1 result already sitting in the accumulator
                for gi, (pad, bb, dst) in enumerate(
                    ((qpad, bq, qh), (kpad, bk, kh), (vpad, bv, vh))
                ):
                    pa = accs[r][gi]
                    paf = pa[:].rearrange("p a b -> p (a b)")[:, 0:DRL]
                    for j, (dy1, dy2, dx) in enumerate(QKV_PAIRS):
                        nc.tensor.matmul(
                            paf, prs[:, gi * 3 + j],
                            _dr_ap(pad, y0, dy1, dy2, dx),
                            start=False, stop=False, perf_mode=drm)
                    for j, (dy, dx) in enumerate(QKV_SINGLES):
                        last = j == len(QKV_SINGLES) - 1
                        nc.tensor.matmul(
                            pa[:, :, PADX : PADX + W], sng[:, gi * 2 + j],
                            _win(pad, y0, dy, dx), start=False, stop=last)
                    pint = pa[:, :, PADX : PADX + W]
                    if gi < 2:
                        e = etmp.tile([C, RPC, W], BF16, tag=f"e{gi}")
                        nc.scalar.activation(e[:], pint, AF.Exp, bias=bb)
                        rl = etmp.tile([C, RPC, W], BF16, tag=f"rl{gi}")
                        nc.vector.tensor_scalar(rl[:], pint, bb, 0.0,
                                                ALU.add, ALU.max)
                        if gi == 1:
                            nc.vector.scalar_tensor_tensor(
                                _rows(dst, r), e[:], 1.0, rl[:],
                                ALU.min, ALU.add,
                                accum_out=ks_parts[:, r : r + 1])
                        else:
                            nc.vector.scalar_tensor_tensor(
                                _rows(dst, r), e[:], 1.0, rl[:],
                                ALU.min, ALU.add)
                    else:
                        nc.vector.tensor_scalar_add(_rows(dst, r), pint, bb)

                # --- lepe PE part
                pl = accL.tile([C, RPC, WP], F32, tag="accl")
                plf = pl[:].rearrange("p a b -> p (a b)")[:, 0:DRL]
                for j, (dy1, dy2, dx) in enumerate(LEP_PE_PAIRS):
                    nc.tensor.matmul(
                        plf, prs[:, 9 + j],
                        _dr_ap(vpad, y0, dy1, dy2, dx),
                        start=(j == 0), stop=False, perf_mode=drm)
                for j, (dy, dx) in enumerate(LEP_PE_SINGLES):
                    last = j == len(LEP_PE_SINGLES) - 1
                    nc.tensor.matmul(
                        pl[:, :, PADX : PADX + W], sng[:, 6 + j],
                        _win(vpad, y0, dy, dx), start=False, stop=last)
                nc.vector.tensor_scalar_add(_rows(lep, r),
                                            pl[:, :, PADX : PADX + W], bl)

                if r % CPQ == CPQ - 1:
                    qi = r // CPQ
                    nc.sync.dma_start_transpose(
                        out=ktf[:, qi * 18 : (qi + 1) * 18, :],
                        in_=kh[qi][:].rearrange("p a b -> p (a b)"))
                    nc.sync.dma_start_transpose(
                        out=vtf[:, qi * 18 : (qi + 1) * 18, :],
                        in_=vh[qi][:].rearrange("p a b -> p (a b)"))

        # ---- kv: 72 accumulating matmuls over buffered token-major tiles
        with tc.tile_pool(name="kvp", bufs=1, space="PSUM") as kvp:
            kvacc = kvp.tile([C, C], F32, tag="kvacc")
            for j in range(72):
                nc.tensor.matmul(kvacc[:], ktf[:, j], vtf[:, j],
                                 start=(j == 0), stop=(j == 71))
            nc.scalar.activation(kv_sb[:], kvacc[:], AF.Copy, scale=float(S2))

        # ---- k_mean -> replicated scaled lhsT
        nc.vector.tensor_reduce(ks_sb[:], ks_parts[:], axis=mybir.AxisListType.X,
                                op=ALU.add)
        nc.vector.tensor_scalar(kmr_sb[:], ones_sb[:], ks_sb[:], ZS,
                                ALU.mult, ALU.mult)

        # ---- phase 4: res per chunk
        with tc.tile_pool(name="ps4", bufs=2, space="PSUM") as ps4, \
             tc.tile_pool(name="psf", bufs=2, space="PSUM") as psf, \
             tc.tile_pool(name="etmp4", bufs=2) as etmp, \
             tc.tile_pool(name="ostg", bufs=2) as ostg:
            for r in range(NCH):
                y0 = r * RPC
                rq = _rows(qh, r)
                rv = _rows(vh, r)
                zp = ps4.tile([C, RPC, W], F32, tag="zp")
                nc.tensor.matmul(zp[:], kmr_sb[:], rq, start=True, stop=True)
                vb = ps4.tile([C, RPC, W], F32, tag="vb")
                nc.tensor.matmul(vb[:], inv_sb[:], rv, start=True, stop=True)
                rp = ps4.tile([C, RPC, W], F32, tag="rp")
                nc.tensor.matmul(rp[:], kv_sb[:], rq, start=True, stop=True)

                zc = etmp.tile([C, RPC, W], F32, tag="zc")
                nc.vector.tensor_scalar(zc[:], zp[:], 1.0, None, ALU.mult)
                rr2 = etmp.tile([C, RPC, W], F32, tag="rr2")
                nc.vector.reciprocal_approx_fast(rr2[:], zc[:])
                vbs = etmp.tile([C, RPC, W], BF16, tag="vbs")
                nc.vector.tensor_scalar(vbs[:], vb[:], 1.0, None, ALU.mult)
                t1 = etmp.tile([C, RPC, W], BF16, tag="t1")
                nc.vector.scalar_tensor_tensor(
                    t1[:], rr2[:], 1.0, rp[:], ALU.add, ALU.mult)
                t2 = etmp.tile([C, RPC, W], BF16, tag="t2")
                nc.vector.tensor_tensor(t2[:], zp[:], vbs[:], ALU.mult)
                r3 = etmp.tile([C, RPC, W], BF16, tag="r3")
                nc.vector.tensor_tensor(r3[:], t1[:], t2[:], ALU.subtract)
                r4 = etmp.tile([C, RPC, W], BF16, tag="r4")
                nc.vector.tensor_tensor(r4[:], r3[:], _rows(lep, r), ALU.add)
                r5 = etmp.tile([C, RPC, W], BF16, tag="r5")
                nc.vector.tensor_tensor(r5[:], r4[:], _rows(of, r), ALU.mult)
                pp = psf.tile([C, RPC, W], F32, tag="pp")
                nc.tensor.matmul(pp[:], wp_sb[:], r5[:], start=True, stop=True)
                if r % 3 == 0:
                    og = ostg.tile([C, 3 * RPC, W], F32, tag="og")
                nc.vector.tensor_scalar_add(
                    og[:, (r % 3) * RPC : (r % 3 + 1) * RPC, :], pp[:], bp)
                if r % 3 == 2:
                    nc.sync.dma_start(
                        out=out_d[b, :, y0 - 2 * RPC : y0 + RPC, :], in_=og[:])


_CACHE = {}


def _get_nc():
    if "nc" not in _CACHE:
        nc = bacc.Bacc("TRN2", target_bir_lowering=False, debug=False)
        with tile.TileContext(nc, pool_alloc_mode="queue") as tc:
            _build(tc)
        nc.compile()
        _CACHE["nc"] = nc
    return _CACHE["nc"]


def _diag(w):
    d = np.zeros((C, C), np.float32)
    np.fill_diagonal(d, w)
    return d


def kernel(**inputs) -> np.ndarray:
    x = np.asarray(inputs["x"], np.float32)
    w_qkvo = np.asarray(inputs["w_qkvo"], np.float32)[:, :, 0, 0]
    b_qkvo = np.asarray(inputs["b_qkvo"], np.float32)
    w_lepe = np.asarray(inputs["w_lepe"], np.float32)[:, 0]
    b_lepe = np.asarray(inputs["b_lepe"], np.float32)
    w_proj = np.asarray(inputs["w_proj"], np.float32)[:, :, 0, 0]
    b_proj = np.asarray(inputs["b_proj"], np.float32)
    w_q = np.asarray(inputs["w_q"], np.float32)[:, 0]
    b_q = np.asarray(inputs["b_q"], np.float32)
    w_k = np.asarray(inputs["w_k"], np.float32)[:, 0]
    b_k = np.asarray(inputs["b_k"], np.float32)
    w_v = np.asarray(inputs["w_v"], np.float32)[:, 0]
    b_v = np.asarray(inputs["b_v"], np.float32)

    # center-tap folding: the 1x1 stationary columns for q/k/v are scaled by
    # (1 + w_center); the pads then hold the scaled 1x1 output, so every tap
    # weight that reads a pad is divided by that channel's center scale.
    cq = 1.0 + w_q[:, 1, 1]
    ck = 1.0 + w_k[:, 1, 1]
    cv = 1.0 + w_v[:, 1, 1]

    pairs = np.zeros((NPAIR, C, 2, C), np.float32)
    sings = np.zeros((NSING, C, C), np.float32)
    for gi, (wt, cs) in enumerate(((w_q, cq), (w_k, ck), (w_v, cv))):
        wt_s = wt / cs[:, None, None]
        for j, (dy1, dy2, dx) in enumerate(QKV_PAIRS):
            pairs[gi * 3 + j, :, 0, :] = _diag(wt_s[:, dy1 + 1, dx + 1])
            pairs[gi * 3 + j, :, 1, :] = _diag(wt_s[:, dy2 + 1, dx + 1])
        for j, (dy, dx) in enumerate(QKV_SINGLES):
            sings[gi * 2 + j] = _diag(wt_s[:, dy + 1, dx + 1])
    wl_s = w_lepe / cv[:, None, None]
    for j, (dy1, dy2, dx) in enumerate(LEP_PE_PAIRS):
        pairs[9 + j, :, 0, :] = _diag(wl_s[:, dy1 + 2, dx + 2])
        pairs[9 + j, :, 1, :] = _diag(wl_s[:, dy2 + 2, dx + 2])
    for j, (dy, dx) in enumerate(LEP_PE_SINGLES):
        sings[6 + j] = _diag(wl_s[:, dy + 2, dx + 2])

    wq_scaled = w_qkvo.copy()           # [4C, C_in]
    wq_scaled[0 * C : 1 * C] *= cq[:, None]
    wq_scaled[1 * C : 2 * C] *= ck[:, None]
    wq_scaled[2 * C : 3 * C] *= cv[:, None]

    bq0, bk0, bv0, bo0 = (b_qkvo[i * C : (i + 1) * C] for i in range(4))
    beff_q = bq0 * (1.0 + w_q.reshape(C, -1).sum(1)) + b_q
    beff_k = bk0 * (1.0 + w_k.reshape(C, -1).sum(1)) + b_k
    beff_v = bv0 * (1.0 + w_v.reshape(C, -1).sum(1)) + b_v
    beff_l = bv0 * w_lepe.reshape(C, -1).sum(1) + b_lepe
    biases = np.stack([beff_q, beff_k, beff_v, bo0, beff_l, b_proj,
                       np.zeros(C, np.float32), np.zeros(C, np.float32)], axis=1)

    if int(os.environ.get("KERNEL_DRSWI", "0")):
        # DoubleRowSwInterleave weight layout: columns reversed, A/B pairs
        # interleaved per column: mem[:, 2j+i] = weight_i[:, 127-j].
        il = np.zeros_like(pairs)                    # [NPAIR, C, 2, C]
        flat = il.reshape(NPAIR, C, 2 * C)
        flat[:, :, 0::2] = pairs[:, :, 0, ::-1]
        flat[:, :, 1::2] = pairs[:, :, 1, ::-1]
        pairs = il

    shared = {
        "wqkvo_t": np.ascontiguousarray(wq_scaled.T),
        "wproj_t": np.ascontiguousarray(w_proj.T),
        "drpairs": np.ascontiguousarray(pairs.transpose(1, 0, 2, 3)).astype(E4),
        "drsing": np.ascontiguousarray(sings.transpose(1, 0, 2)).astype(E4),
        "biases": biases.astype(np.float32),
    }
    xb = x.reshape(NCORES, BPC, C, H, W)
    in_maps = [{"x": np.ascontiguousarray(xb[i]), **shared} for i in range(NCORES)]

    nc = _get_nc()
    _CACHE["last_in_maps"] = in_maps
    r = run_bass_kernel_spmd(
        nc, in_maps, core_ids=list(range(NCORES)),
        trace=bool(int(os.environ.get("KERNEL_TRACE", "0"))),
    )
    _CACHE["last_results"] = r
    out = np.stack([r.results[i]["out"] for i in range(NCORES)])
    return out.reshape(B, C, H, W)
